# revision 2
# baseline (speedup 1.0000x reference)
"""RBF Nadaraya-Watson regression kernel for Trainium2, 8-core SPMD.

out = (K @ Ytrain) / (sum_j K + EPS),  K = exp(-||xt - xj||^2 / (2 l^2))

Sharding: Xtest rows split across 8 cores; each core holds full
Xtrain/Ytrain and computes its slice independently (no collectives).

Host-side prep (layout/dtype only, no math): Xtrain is passed both
transposed+fp8 (gram weights, removes on-device PE transposes + casts)
and bf16 (row norms); Ytrain is passed fp8 padded to 80 cols with a
ones column at 64 (denominator).

Per-core algorithm (T = Ntest/8 = 1024 test rows):
  a_t = ||xtest_t||^2 (fp32), s = 0.5*exp(-2*theta)
  For each 128-row train chunk j:
    b_j   = ||xtrain_j||^2 (bf16 squares, fp32 row-sums, per superchunk)
    G^T[j, t] = sum_d Xtrain[j,d] Xtest[t,d]     (PE, fp8 DoubleRow)
    K0^T[j,t] = exp(2s*G - s*b_j) as fp8, via either
      - ACT: exp activation (bias per partition), or
      - DVE: Schraudolph bit-trick exp: byte = rne(A*x + B) with
        A = 8*log2(e), B = 55.63; uint8 saturation at 0 clamps
        underflow; byte IS the fp8e4m3 encoding of ~exp(x) (+-7%).
      The split keeps both engines busy (ACT is the bottleneck).
    numer0^T[y, t] += sum_j [Y | 1][j,y] K0^T[j,t]  (PE, fp8 DoubleRow)
  out[t,:] = numer0[t, 0:64] / (numer0[t, 64] + EPS*exp(s*a_t))
The a_t term is folded multiplicatively: mathematically identical to
exp(-s(a+b) + 2s c) / (sum exp(...) + EPS) in real arithmetic.
"""

import sys

try:
    import concourse.bass as bass  # noqa: F401
except ImportError:
    sys.path.insert(0, "/opt/trn_rl_repo")

import numpy as np
import ml_dtypes

import concourse.bass as bass
import concourse.bacc as bacc
import concourse.tile as tile
from concourse import mybir
from concourse.bass_utils import run_bass_kernel_spmd

AF = mybir.ActivationFunctionType
ALU = mybir.AluOpType
F32 = mybir.dt.float32
BF16 = mybir.dt.bfloat16
FP8 = mybir.dt.float8e4
U8 = mybir.dt.uint8

LNEPS = float(np.log(1e-8))
# Schraudolph constants for fp8e4m3 bit-pattern exp
SCHR_A = float(8.0 / np.log(2.0))   # 11.5416
SCHR_B = 55.63                       # 56 + 8*c, c calibrated on HW semantics

# chunks per superchunk (of 8) whose exp runs on DVE instead of ACT
SCHR_PATTERNS = {
    0: (),
    1: (3,),
    2: (1, 5),
    3: (1, 4, 6),
    4: (0, 2, 4, 6),
    5: (0, 2, 3, 5, 7),
    6: (0, 1, 2, 4, 5, 6),
    8: tuple(range(8)),
}

BUILD_KW = {}


def build(T=1024, NTRAIN=32768, D=256, DY=64, SUPER=8, schr=3, reps=1):
    """Build the per-core Bass module. T = test rows per core.

    schr: how many of each 8 chunks compute exp on DVE (Schraudolph)
    instead of ACT. reps > 1 repeats the computation for differential
    device-time measurement.
    """
    assert T % 128 == 0 and NTRAIN % (128 * SUPER) == 0 and D == 256
    NCHUNK = NTRAIN // 128
    TT = T // 128
    segs = [(s, min(s + 512, T)) for s in range(0, T, 512)]
    DYP = DY + 1
    YPAD = 80
    offload = set(SCHR_PATTERNS[schr])

    nc = bacc.Bacc("TRN2", target_bir_lowering=False, debug=False)
    xtrainT8_d = nc.dram_tensor("xtrainT8", [D, NTRAIN], FP8,
                                kind="ExternalInput")
    xtrainb_d = nc.dram_tensor("xtrainb", [NTRAIN, D], BF16,
                               kind="ExternalInput")
    ypad8_d = nc.dram_tensor("ypad8", [NTRAIN, YPAD], FP8,
                             kind="ExternalInput")
    xtest_d = nc.dram_tensor("Xtest", [T, D], F32, kind="ExternalInput")
    theta_d = nc.dram_tensor("theta", [1, 1], F32, kind="ExternalInput")
    identf_d = nc.dram_tensor("identf", [128, 128], F32, kind="ExternalInput")
    out_d = nc.dram_tensor("out", [T, DY], F32, kind="ExternalOutput")

    with tile.TileContext(nc) as tc:
      for _rep in range(reps):
        with (
            tc.tile_pool(name="persist", bufs=1) as persist,
            tc.tile_pool(name="xstage", bufs=2) as xstage,
            tc.tile_pool(name="xtT8p", bufs=2) as xtT8p,
            tc.tile_pool(name="xcbp", bufs=2) as xcbp,
            tc.tile_pool(name="y8p", bufs=2) as y8p,
            tc.tile_pool(name="sqp", bufs=2) as sqp,
            tc.tile_pool(name="k0tp", bufs=3) as k0tp,
            tc.tile_pool(name="biasp", bufs=4) as biasp,
            tc.tile_pool(name="epi", bufs=2) as epi,
            tc.tile_pool(name="gp_pool", bufs=2, space="PSUM") as gp_pool,
            tc.tile_pool(name="np_pool", bufs=1, space="PSUM") as np_pool,
            tc.tile_pool(name="tp_pool", bufs=2, space="PSUM") as tp_pool,
        ):
            # ---- constants / scalars ----
            identf = persist.tile([128, 128], F32)
            nc.sync.dma_start(identf[:], identf_d.ap())
            theta = persist.tile([1, 1], F32)
            nc.sync.dma_start(theta[:], theta_d.ap())

            sv = persist.tile([1, 4], F32)
            # sv = [2s, s, -s, -A*s] with s = 0.5*exp(-2*theta)
            nc.scalar.activation(sv[0:1, 0:1], theta[:], AF.Exp, scale=-2.0)
            nc.vector.tensor_scalar_mul(sv[0:1, 1:2], sv[0:1, 0:1], 0.5)
            nc.vector.tensor_scalar_mul(sv[0:1, 2:3], sv[0:1, 0:1], -0.5)
            nc.vector.tensor_scalar_mul(sv[0:1, 3:4], sv[0:1, 0:1],
                                        -0.5 * SCHR_A)

            ones_row = persist.tile([1, 128], F32)
            nc.vector.memset(ones_row[:], 1.0)
            bc_ps = tp_pool.tile([128, 4], F32, tag="t")
            nc.tensor.matmul(bc_ps[:], lhsT=ones_row[:], rhs=sv[0:1, 0:4])
            sbc = persist.tile([128, 4], F32)
            nc.vector.tensor_copy(sbc[:], bc_ps[:])
            s2_vec = sbc[:, 0:1]   # 2s broadcast on partitions
            s_vec = sbc[:, 1:2]    # s
            ms_vec = sbc[:, 2:3]   # -s
            msA_vec = sbc[:, 3:4]  # -A*s
            sA2 = persist.tile([128, 1], F32)
            nc.vector.tensor_scalar_mul(sA2[:], s2_vec, SCHR_A)  # A*2s

            # ---- Xtest: a_t norms + transposed fp8 copies ----
            xtestT8 = persist.tile([128, D // 128, T], FP8)
            a8 = persist.tile([128, TT], F32)
            for tt in range(TT):
                xts = xstage.tile([128, D], F32, tag="xts")
                nc.sync.dma_start(xts[:], xtest_d.ap()[tt * 128:(tt + 1) * 128, :])
                sqs = sqp.tile([128, D], F32, tag="sqf", name="sqf")
                nc.vector.tensor_mul(sqs[:], xts[:], xts[:])
                nc.vector.reduce_sum(a8[:, tt:tt + 1], sqs[:],
                                     axis=mybir.AxisListType.X)
                for k in range(D // 128):
                    tps = tp_pool.tile([128, 128], F32, tag="t", name="tps")
                    nc.tensor.transpose(tps[:], xts[:, k * 128:(k + 1) * 128],
                                        identf[:])
                    nc.vector.tensor_copy(
                        xtestT8[:, k, tt * 128:(tt + 1) * 128], tps[:])

            sa8 = persist.tile([128, TT], F32)
            nc.vector.tensor_scalar_mul(sa8[:], a8[:], s_vec)
            lneps_t = persist.tile([128, 1], F32)
            nc.vector.memset(lneps_t[:], LNEPS)
            epst8 = persist.tile([128, TT], F32)
            # EPS * exp(s*a_t) = exp(s*a_t + ln(EPS))
            nc.scalar.activation(epst8[:], sa8[:], AF.Exp, bias=lneps_t[:])

            # ---- main loop over train superchunks ----
            np_ps = np_pool.tile([DYP, T], F32)
            nsuper = NCHUNK // SUPER
            for c0 in range(nsuper):
                r0 = c0 * SUPER * 128
                # transposed fp8 gram weights [128, 2, SUPER*128]
                xtT8 = xtT8p.tile([128, D // 128, SUPER * 128], FP8,
                                  tag="xtT8")
                nc.sync.dma_start(
                    xtT8[:],
                    xtrainT8_d.ap()[:, r0:r0 + SUPER * 128].rearrange(
                        "(k p) j -> p k j", p=128))
                # bf16 rows for norms [128, SUPER, D]
                xcb = xcbp.tile([128, SUPER, D], BF16, tag="xcb")
                nc.sync.dma_start(
                    xcb[:],
                    xtrainb_d.ap()[r0:r0 + SUPER * 128, :].rearrange(
                        "(c p) d -> p c d", p=128))
                # fp8 Y (+ones col at DY) [128, SUPER, YPAD]
                y8 = y8p.tile([128, SUPER, YPAD], FP8, tag="y8")
                nc.sync.dma_start(
                    y8[:],
                    ypad8_d.ap()[r0:r0 + SUPER * 128, :].rearrange(
                        "(c p) y -> p c y", p=128))

                # norms: b8[p, c] = sum_d xcb^2 ; biases for both exp paths
                sq8 = sqp.tile([128, SUPER, D], BF16, tag="sq8", name="sq8")
                nc.vector.tensor_mul(sq8[:], xcb[:], xcb[:])
                b8 = biasp.tile([128, SUPER], F32, tag="b8", name="b8")
                nc.vector.reduce_sum(b8[:], sq8[:], axis=mybir.AxisListType.X)
                bias8 = biasp.tile([128, SUPER], F32, tag="bias8",
                                   name="bias8")
                nc.vector.tensor_scalar_mul(bias8[:], b8[:], ms_vec)
                if offload:
                    bimm8 = biasp.tile([128, SUPER], F32, tag="bimm8",
                                       name="bimm8")
                    nc.vector.tensor_scalar(bimm8[:], b8[:], msA_vec[:, 0:1],
                                            SCHR_B, op0=ALU.mult, op1=ALU.add)

                for cc in range(SUPER):
                    c = c0 * SUPER + cc
                    # gram: G^T[j, t] for this chunk (fp8 DoubleRow)
                    gp = gp_pool.tile([128, T], F32, tag="g", name="gp")
                    for (s0, s1) in segs:
                        nc.tensor.matmul(
                            gp[:, s0:s1],
                            lhsT=xtT8[:, :, cc * 128:(cc + 1) * 128],
                            rhs=xtestT8[:, :, s0:s1],
                            perf_mode=mybir.MatmulPerfMode.DoubleRow,
                        )

                    # K0^T = exp(2s*G - s*b_j) as fp8 (bits in a u8 tile)
                    if cc % 2 == 0:
                        k0t2 = k0tp.tile([128, 2, T], U8, tag="k0t",
                                         name="k0t2")
                    dst = k0t2[:, cc % 2, :]
                    if cc in offload:
                        # DVE: byte = rne(A*2s*G + (B - A*s*b_j)), sat at 0
                        nc.vector.tensor_scalar(
                            dst, gp[:], sA2[:, 0:1], bimm8[:, cc:cc + 1],
                            op0=ALU.mult, op1=ALU.add)
                    else:
                        nc.scalar.activation(
                            dst.bitcast(FP8), gp[:], AF.Exp,
                            bias=bias8[:, cc:cc + 1], scale=s2_vec)

                    # numer0^T[y, t] accumulation per chunk pair
                    if cc % 2 == 1:
                        for (s0, s1) in segs:
                            nc.tensor.matmul(
                                np_ps[:, s0:s1],
                                lhsT=y8[:, cc - 1:cc + 1, 0:DYP],
                                rhs=k0t2[:, :, s0:s1].bitcast(FP8),
                                perf_mode=mybir.MatmulPerfMode.DoubleRow,
                                start=(c == 1),
                                stop=(c == NCHUNK - 1),
                                skip_group_check=True,
                            )

            # ---- epilogue: transpose numer^T, divide, store ----
            ncopy = epi.tile([DYP, T], F32, bufs=1)
            nc.vector.tensor_copy(ncopy[:], np_ps[:])
            for tt in range(TT):
                ntp = tp_pool.tile([128, DYP], F32, tag="t", name="ntp")
                nc.tensor.transpose(
                    ntp[:], ncopy[:, tt * 128:(tt + 1) * 128],
                    identf[0:DYP, 0:DYP])
                dvec = biasp.tile([128, 1], F32, tag="dv", name="dvec")
                nc.vector.tensor_add(dvec[:], ntp[:, DY:DYP],
                                     epst8[:, tt:tt + 1])
                rvec = biasp.tile([128, 1], F32, tag="rv", name="rvec")
                nc.vector.reciprocal(rvec[:], dvec[:])
                otile = epi.tile([128, DY], F32, tag="o", name="otile")
                nc.vector.tensor_scalar_mul(otile[:], ntp[:, 0:DY], rvec[:])
                nc.sync.dma_start(out_d.ap()[tt * 128:(tt + 1) * 128, :],
                                  otile[:])

    nc.compile()
    return nc


_NC_CACHE = {}


def _get_nc(T, NTRAIN, D, DY):
    key = (T, NTRAIN, D, DY)
    if key not in _NC_CACHE:
        _NC_CACHE[key] = build(T=T, NTRAIN=NTRAIN, D=D, DY=DY, **BUILD_KW)
    return _NC_CACHE[key]


def make_in_maps(Ytrain, Xtrain, Xtest, log_lengthscale, n_cores=8):
    Xtrain = np.ascontiguousarray(np.asarray(Xtrain, dtype=np.float32))
    Ytrain = np.ascontiguousarray(np.asarray(Ytrain, dtype=np.float32))
    Xtest = np.ascontiguousarray(np.asarray(Xtest, dtype=np.float32))
    theta = np.asarray(log_lengthscale, dtype=np.float32).reshape(1, 1)
    identf = np.eye(128, dtype=np.float32)

    xtrainT8 = np.ascontiguousarray(
        Xtrain.T.astype(ml_dtypes.float8_e4m3))
    xtrainb = Xtrain.astype(ml_dtypes.bfloat16)
    ntrain, dy = Ytrain.shape
    ypad = np.zeros((ntrain, 80), dtype=np.float32)
    ypad[:, :dy] = Ytrain
    ypad[:, dy] = 1.0
    ypad8 = ypad.astype(ml_dtypes.float8_e4m3)

    shards = np.split(Xtest, n_cores, axis=0)
    return [
        {
            "xtrainT8": xtrainT8,
            "xtrainb": xtrainb,
            "ypad8": ypad8,
            "Xtest": shards[i],
            "theta": theta,
            "identf": identf,
        }
        for i in range(n_cores)
    ]


def kernel(Ytrain, Xtrain, Xtest, log_lengthscale):
    n_cores = 8
    ntest, d = np.asarray(Xtest).shape
    ntrain, dy = np.asarray(Ytrain).shape
    nc = _get_nc(ntest // n_cores, ntrain, d, dy)
    in_maps = make_in_maps(Ytrain, Xtrain, Xtest, log_lengthscale, n_cores)
    res = run_bass_kernel_spmd(nc, in_maps, core_ids=list(range(n_cores)))
    return np.concatenate([res.results[i]["out"] for i in range(n_cores)],
                          axis=0)
